# revision 20
# baseline (speedup 1.0000x reference)
"""MoE-LoRA linear kernel for Trainium2 (8 NeuronCores, data-parallel over tokens).

Computes, for x:[B,S,Din], base_w:[Dout,Din], gate_w:[E,Din],
lora_A:[E*R,Din], lora_B:[Dout,E*R]:

    base   = x @ base_w.T
    logits = x @ gate_w.T ; top-2 renormalized softmax -> dense w:[*,E]
    ax     = x @ lora_A.T                 (per-expert rank-R blocks)
    delta  = (ax * w_expanded) @ lora_B.T * SCALING
    out    = base + delta

Sharding: tokens (B*S=8192) split across 8 cores, 1024 tokens each.
Weights replicated. No collectives.

On-chip per core:
  phase 1a: x streamed once as fp32 [d, t-tile]; gating logits in true fp32
            (top-2 via DVE Max8, renormalized via sigmoid identity, dense
            weights via equality masks); each x tile then copied on-chip to
            the persistent fp32r x buffer feeding every other matmul.
  phase 1b: ax in fp32r (full PE speed), gate-weight multiply, PE transpose
            to [r, t] layout for the delta matmul.
  phase 2:  per 512-wide output tile: 32 base matmuls + 4 delta matmuls
            accumulate into one PSUM bank, copy out. All fp32r.

SCALING is folded into lora_B host-side.
"""
import sys

if "/opt/trn_rl_repo" not in sys.path:
    sys.path.insert(0, "/opt/trn_rl_repo")

import numpy as np

import concourse.bacc as bacc
import concourse.mybir as mybir
import concourse.tile as tile
from concourse import bass_utils
from concourse.bass import ds, ts

B, S, DIN, DOUT = 4, 2048, 4096, 4096
E, R = 32, 16
SCALING = 2.0
NCORES = 8
T = (B * S) // NCORES  # 1024 tokens per core
P = 128
TT = T // P            # 8 token tiles
KT = DIN // P          # 32 contraction tiles
OT = DOUT // 512       # 8 output column tiles
RR = (E * R) // P      # 4 rank tiles
KC = 16                # base-weight chunks per o-tile (2 k-slices each)
F32 = mybir.dt.float32
F32R = mybir.dt.float32r

_CACHE = {}


def _build():
    nc = bacc.Bacc("TRN2", target_bir_lowering=False, debug=False)
    xT = nc.dram_tensor("xT", [DIN, T], F32, kind="ExternalInput")
    bwT = nc.dram_tensor("bwT", [DIN, DOUT], F32R, kind="ExternalInput")
    gwT = nc.dram_tensor("gwT", [DIN, E], F32, kind="ExternalInput")
    laT = nc.dram_tensor("laT", [DIN, E * R], F32R, kind="ExternalInput")
    lbT = nc.dram_tensor("lbT", [E * R, DOUT], F32R, kind="ExternalInput")
    iden = nc.dram_tensor("iden", [P, P], F32R, kind="ExternalInput")
    out = nc.dram_tensor("out", [T, DOUT], F32, kind="ExternalOutput")

    xT3 = xT.ap().rearrange("(k p) t -> p k t", p=P)
    gwT3 = gwT.ap().rearrange("(k p) e -> p k e", p=P)
    laT3 = laT.ap().rearrange("(k p) r -> p k r", p=P)
    lbT3 = lbT.ap().rearrange("(rr p) o -> p rr o", p=P)
    bwT2 = bwT.ap()
    out2 = out.ap()

    with tile.TileContext(nc, pool_alloc_mode="queue") as tc:
        with (
            tc.tile_pool(name="base", bufs=1) as bp,
            tc.tile_pool(name="psum", bufs=8, space="PSUM") as psum,
        ):
            identity = bp.tile([P, P], F32R, tag="iden")
            xsb = bp.tile([P, KT, T], F32R, tag="xsb")
            axwT = bp.tile([P, RR, T], F32R, tag="axwT")
            wdense = []
            for t in range(TT):
                wd = bp.tile([P, E], F32, tag=f"wd{t}", name=f"wd{t}")
                wdense.append(wd)

            # ---- phase 1a: stream x once (fp32); gating + on-chip fp32r copy
            with tc.tile_pool(name="p1a", bufs=2) as p1a:
                KH = KT // 2
                gwt = p1a.tile([P, KT, E], F32, tag="gw", bufs=1)
                nc.sync.dma_start(gwt[:, :KH, :], gwT3[:, :KH, :])
                gw_hi_loaded = False
                for t in range(TT):
                    pl = psum.tile([P, E], F32, tag="bank", name="pl")
                    for h in range(2):
                        x32 = p1a.tile(
                            [P, KH, P], F32, tag="x32", name="x32", bufs=3
                        )
                        nc.sync.dma_start(
                            x32[:], xT3[:, ds(h * KH, KH), ts(t, P)]
                        )
                        if not gw_hi_loaded:
                            nc.sync.dma_start(
                                gwt[:, KH:, :], gwT3[:, KH:, :]
                            )
                            gw_hi_loaded = True
                        for k in range(KH):
                            nc.tensor.matmul(
                                pl[:], x32[:, k, :], gwt[:, h * KH + k, :],
                                start=(h == 0 and k == 0),
                                stop=(h == 1 and k == KH - 1),
                            )
                        # persist the fp32r copy for all later matmuls
                        # (GPSIMD: keeps DVE free so the x32 slot recycles
                        # without stalling the next tile's DMA)
                        nc.gpsimd.tensor_copy(
                            xsb[:, ds(h * KH, KH), ts(t, P)],
                            x32[:].bitcast(F32R),
                        )
                    lsb = p1a.tile([P, E], F32, tag="lsb", name="lsb")
                    nc.vector.tensor_copy(lsb[:], pl[:])
                    m8 = p1a.tile([P, 8], F32, tag="m8", name="m8")
                    nc.vector.max(out=m8[:], in_=lsb[:])
                    d21 = p1a.tile([P, 1], F32, tag="d21", name="d21")
                    nc.vector.tensor_sub(d21[:], m8[:, 1:2], m8[:, 0:1])
                    e2 = p1a.tile([P, 1], F32, tag="e2", name="e2")
                    nc.scalar.activation(
                        e2[:], d21[:], mybir.ActivationFunctionType.Exp
                    )
                    den = p1a.tile([P, 1], F32, tag="den", name="den")
                    nc.vector.tensor_scalar_add(den[:], e2[:], 1.0)
                    w1 = p1a.tile([P, 1], F32, tag="w1", name="w1")
                    nc.vector.reciprocal(w1[:], den[:])
                    w2 = p1a.tile([P, 1], F32, tag="w2", name="w2")
                    nc.vector.tensor_mul(w2[:], e2[:], w1[:])
                    eq1 = p1a.tile([P, E], F32, tag="eq1", name="eq1")
                    nc.vector.tensor_tensor(
                        eq1[:], lsb[:], m8[:, 0:1].to_broadcast([P, E]),
                        mybir.AluOpType.is_equal,
                    )
                    eq2 = p1a.tile([P, E], F32, tag="eq2", name="eq2")
                    nc.vector.tensor_tensor(
                        eq2[:], lsb[:], m8[:, 1:2].to_broadcast([P, E]),
                        mybir.AluOpType.is_equal,
                    )
                    nc.vector.tensor_tensor(
                        eq1[:], eq1[:], w1[:].to_broadcast([P, E]),
                        mybir.AluOpType.mult,
                    )
                    nc.vector.tensor_tensor(
                        eq2[:], eq2[:], w2[:].to_broadcast([P, E]),
                        mybir.AluOpType.mult,
                    )
                    nc.vector.tensor_add(wdense[t][:], eq1[:], eq2[:])

            # ---- phase 1b: ax (fp32r), gate multiply, transpose ----
            nc.sync.dma_start(identity[:], iden.ap())
            with tc.tile_pool(name="p1b", bufs=2) as p1b:
                axps = []
                for t in range(TT):
                    ap_t = psum.tile(
                        [P, 512], F32, tag="bank", name=f"axps{t}"
                    )
                    axps.append(ap_t)
                for k in range(KT):
                    lak = p1b.tile([P, 512], F32R, tag="lak", name="lak", bufs=4)
                    nc.sync.dma_start(lak[:], laT3[:, k, :])
                    for t in range(TT):
                        nc.tensor.matmul(
                            axps[t][:], xsb[:, k, ts(t, P)], lak[:],
                            start=(k == 0), stop=(k == KT - 1),
                        )
                axws = []
                for t in range(TT):
                    axw = p1b.tile(
                        [P, 512], F32R, tag=f"axw{t}", name=f"axw{t}", bufs=1
                    )
                    nc.vector.tensor_tensor(
                        axw[:].rearrange("p (e r) -> p e r", r=R),
                        axps[t][:].rearrange("p (e r) -> p e r", r=R),
                        wdense[t][:, :, None].to_broadcast([P, E, R]),
                        mybir.AluOpType.mult,
                    )
                    axws.append(axw)
                for t in range(TT):
                    tpq = psum.tile([P, 512], F32R, tag="bank", name="tpq")
                    for rr in range(RR):
                        nc.tensor.transpose(
                            tpq[:, ts(rr, P)], axws[t][:, ts(rr, P)],
                            identity[:],
                        )
                    nc.vector.tensor_copy(
                        axwT[:, :, ts(t, P)],
                        tpq[:].rearrange("p (rr q) -> p rr q", q=P),
                    )

            # ---- phase 2: base + delta per output tile ----
            KPC = KT // KC  # k-slices per base-weight chunk
            with (
                tc.tile_pool(name="p2bw", bufs=4) as p2bw,
                tc.tile_pool(name="p2lb", bufs=2) as p2lb,
                tc.tile_pool(name="p2o", bufs=2) as p2o,
            ):

                def load_lb(o):
                    lb = p2lb.tile([P, RR, 512], F32R, tag="lb", name="lb")
                    nc.sync.dma_start(lb[:], lbT3[:, :, ds(o * 512, 512)])
                    return lb

                def load_bwc(o, kc):
                    bwc = p2bw.tile([P, KPC, 512], F32R, tag="bwc", name="bwc")
                    nc.sync.dma_start(
                        bwc[:],
                        bwT2[
                            ds(kc * KPC * P, KPC * P), ds(o * 512, 512)
                        ].rearrange("(kk p) o -> p kk o", p=P),
                    )
                    return bwc

                lb_next = load_lb(0)
                bw_pre = {0: load_bwc(0, 0), 1: load_bwc(0, 1)}
                for o in range(OT):
                    lb = lb_next
                    ps2 = {}
                    for kc in range(KC):
                        bwc = bw_pre.pop(kc, None)
                        if bwc is None:
                            bwc = load_bwc(o, kc)
                        for t in range(TT):
                            if kc == 0:
                                ps2[t] = psum.tile(
                                    [P, 512], F32, tag="bank",
                                    name=f"ps2_{o}_{t}",
                                )
                            for k in range(KPC):
                                nc.tensor.matmul(
                                    ps2[t][:],
                                    xsb[:, kc * KPC + k, ts(t, P)],
                                    bwc[:, k, :],
                                    start=(kc == 0 and k == 0),
                                    stop=False,
                                )
                    # prefetch next o ahead of this o's output burst
                    if o + 1 < OT:
                        lb_next = load_lb(o + 1)
                        bw_pre = {
                            0: load_bwc(o + 1, 0),
                            1: load_bwc(o + 1, 1),
                        }
                    for t in range(TT):
                        for rr in range(RR):
                            nc.tensor.matmul(
                                ps2[t][:],
                                axwT[:, rr, ts(t, P)],
                                lb[:, rr, :],
                                start=False,
                                stop=(rr == RR - 1),
                            )
                        osb = p2o.tile([P, 512], F32, tag="osb", name="osb")
                        nc.vector.tensor_copy(osb[:], ps2[t][:])
                        nc.sync.dma_start(
                            out2[ts(t, P), ds(o * 512, 512)], osb[:]
                        )

    nc.compile()
    return nc


def _get_nc():
    if "nc" not in _CACHE:
        _CACHE["nc"] = _build()
    return _CACHE["nc"]


def kernel(x, base_w, gate_w, lora_A, lora_B):
    nc = _get_nc()

    x2 = np.ascontiguousarray(np.asarray(x, dtype=np.float32).reshape(B * S, DIN))
    bwT = np.ascontiguousarray(np.asarray(base_w, dtype=np.float32).T)
    gwT = np.ascontiguousarray(np.asarray(gate_w, dtype=np.float32).T)
    laT = np.ascontiguousarray(np.asarray(lora_A, dtype=np.float32).T)
    lbT = np.ascontiguousarray(
        np.asarray(lora_B, dtype=np.float32).T * np.float32(SCALING)
    )
    iden = np.eye(P, dtype=np.float32)

    in_maps = []
    for c in range(NCORES):
        xT_c = np.ascontiguousarray(x2[c * T : (c + 1) * T].T)
        in_maps.append(
            {
                "xT": xT_c,
                "bwT": bwT,
                "gwT": gwT,
                "laT": laT,
                "lbT": lbT,
                "iden": iden,
            }
        )

    res = bass_utils.run_bass_kernel_spmd(nc, in_maps, core_ids=list(range(NCORES)))
    parts = [res.results[c]["out"] for c in range(NCORES)]
    return np.concatenate(parts, axis=0).reshape(B, S, DOUT).astype(np.float32)


# revision 21
# speedup vs baseline: 1.0227x; 1.0227x over previous
"""MoE-LoRA linear kernel for Trainium2 (8 NeuronCores, data-parallel over tokens).

Computes, for x:[B,S,Din], base_w:[Dout,Din], gate_w:[E,Din],
lora_A:[E*R,Din], lora_B:[Dout,E*R]:

    base   = x @ base_w.T
    logits = x @ gate_w.T ; top-2 renormalized softmax -> dense w:[*,E]
    ax     = x @ lora_A.T                 (per-expert rank-R blocks)
    delta  = (ax * w_expanded) @ lora_B.T * SCALING
    out    = base + delta

Sharding: tokens (B*S=8192) split across 8 cores, 1024 tokens each.
Weights replicated. No collectives.

On-chip per core:
  phase 1a: x streamed once as fp32 [d, t-tile]; gating logits in true fp32
            (top-2 via DVE Max8, renormalized via sigmoid identity, dense
            weights via equality masks); each x tile then copied on-chip to
            the persistent fp32r x buffer feeding every other matmul.
  phase 1b: ax in fp32r (full PE speed), gate-weight multiply, PE transpose
            to [r, t] layout for the delta matmul.
  phase 2:  per 512-wide output tile: 32 base matmuls + 4 delta matmuls
            accumulate into one PSUM bank, copy out. All fp32r.

SCALING is folded into lora_B host-side.
"""
import sys

if "/opt/trn_rl_repo" not in sys.path:
    sys.path.insert(0, "/opt/trn_rl_repo")

import numpy as np

import concourse.bacc as bacc
import concourse.mybir as mybir
import concourse.tile as tile
from concourse import bass_utils
from concourse.bass import ds, ts

B, S, DIN, DOUT = 4, 2048, 4096, 4096
E, R = 32, 16
SCALING = 2.0
NCORES = 8
T = (B * S) // NCORES  # 1024 tokens per core
P = 128
TT = T // P            # 8 token tiles
KT = DIN // P          # 32 contraction tiles
OT = DOUT // 512       # 8 output column tiles
RR = (E * R) // P      # 4 rank tiles
KC = 16                # base-weight chunks per o-tile (2 k-slices each)
F32 = mybir.dt.float32
F32R = mybir.dt.float32r

_CACHE = {}


def _build():
    nc = bacc.Bacc("TRN2", target_bir_lowering=False, debug=False)
    xT = nc.dram_tensor("xT", [DIN, T], F32, kind="ExternalInput")
    bwT = nc.dram_tensor("bwT", [DIN, DOUT], F32R, kind="ExternalInput")
    gwT = nc.dram_tensor("gwT", [DIN, E], F32, kind="ExternalInput")
    laT = nc.dram_tensor("laT", [DIN, E * R], F32R, kind="ExternalInput")
    lbT = nc.dram_tensor("lbT", [E * R, DOUT], F32R, kind="ExternalInput")
    iden = nc.dram_tensor("iden", [P, P], F32R, kind="ExternalInput")
    out = nc.dram_tensor("out", [T, DOUT], F32, kind="ExternalOutput")

    xT3 = xT.ap().rearrange("(k p) t -> p k t", p=P)
    gwT3 = gwT.ap().rearrange("(k p) e -> p k e", p=P)
    laT3 = laT.ap().rearrange("(k p) r -> p k r", p=P)
    lbT3 = lbT.ap().rearrange("(rr p) o -> p rr o", p=P)
    bwT2 = bwT.ap()
    out2 = out.ap()

    with tile.TileContext(nc, pool_alloc_mode="queue") as tc:
        with (
            tc.tile_pool(name="base", bufs=1) as bp,
            tc.tile_pool(name="psum", bufs=8, space="PSUM") as psum,
        ):
            identity = bp.tile([P, P], F32R, tag="iden")
            xsb = bp.tile([P, KT, T], F32R, tag="xsb")
            axwT = bp.tile([P, RR, T], F32R, tag="axwT")
            wdense = []
            for t in range(TT):
                wd = bp.tile([P, E], F32, tag=f"wd{t}", name=f"wd{t}")
                wdense.append(wd)

            # ---- phase 1a: stream x once (fp32); gating + on-chip fp32r copy
            with tc.tile_pool(name="p1a", bufs=2) as p1a:
                KH = KT // 2
                gwt = p1a.tile([P, KT, E], F32, tag="gw", bufs=1)
                nc.sync.dma_start(gwt[:, :KH, :], gwT3[:, :KH, :])
                gw_hi_loaded = False
                for t in range(TT):
                    pl = psum.tile([P, E], F32, tag="bank", name="pl")
                    for h in range(2):
                        x32 = p1a.tile(
                            [P, KH, P], F32, tag="x32", name="x32", bufs=3
                        )
                        nc.sync.dma_start(
                            x32[:], xT3[:, ds(h * KH, KH), ts(t, P)]
                        )
                        if not gw_hi_loaded:
                            nc.sync.dma_start(
                                gwt[:, KH:, :], gwT3[:, KH:, :]
                            )
                            gw_hi_loaded = True
                        for k in range(KH):
                            nc.tensor.matmul(
                                pl[:], x32[:, k, :], gwt[:, h * KH + k, :],
                                start=(h == 0 and k == 0),
                                stop=(h == 1 and k == KH - 1),
                            )
                        # persist the fp32r copy for all later matmuls
                        # (GPSIMD: keeps DVE free so the x32 slot recycles
                        # without stalling the next tile's DMA)
                        nc.gpsimd.tensor_copy(
                            xsb[:, ds(h * KH, KH), ts(t, P)],
                            x32[:].bitcast(F32R),
                        )
                    lsb = p1a.tile([P, E], F32, tag="lsb", name="lsb")
                    nc.vector.tensor_copy(lsb[:], pl[:])
                    m8 = p1a.tile([P, 8], F32, tag="m8", name="m8")
                    nc.vector.max(out=m8[:], in_=lsb[:])
                    d21 = p1a.tile([P, 1], F32, tag="d21", name="d21")
                    nc.vector.tensor_sub(d21[:], m8[:, 1:2], m8[:, 0:1])
                    e2 = p1a.tile([P, 1], F32, tag="e2", name="e2")
                    nc.scalar.activation(
                        e2[:], d21[:], mybir.ActivationFunctionType.Exp
                    )
                    den = p1a.tile([P, 1], F32, tag="den", name="den")
                    nc.vector.tensor_scalar_add(den[:], e2[:], 1.0)
                    w1 = p1a.tile([P, 1], F32, tag="w1", name="w1")
                    nc.vector.reciprocal(w1[:], den[:])
                    w2 = p1a.tile([P, 1], F32, tag="w2", name="w2")
                    nc.vector.tensor_mul(w2[:], e2[:], w1[:])
                    eq1 = p1a.tile([P, E], F32, tag="eq1", name="eq1")
                    nc.vector.tensor_tensor(
                        eq1[:], lsb[:], m8[:, 0:1].to_broadcast([P, E]),
                        mybir.AluOpType.is_equal,
                    )
                    eq2 = p1a.tile([P, E], F32, tag="eq2", name="eq2")
                    nc.vector.tensor_tensor(
                        eq2[:], lsb[:], m8[:, 1:2].to_broadcast([P, E]),
                        mybir.AluOpType.is_equal,
                    )
                    nc.vector.tensor_tensor(
                        eq1[:], eq1[:], w1[:].to_broadcast([P, E]),
                        mybir.AluOpType.mult,
                    )
                    nc.vector.tensor_tensor(
                        eq2[:], eq2[:], w2[:].to_broadcast([P, E]),
                        mybir.AluOpType.mult,
                    )
                    nc.vector.tensor_add(wdense[t][:], eq1[:], eq2[:])

            # ---- phase 1b: ax (fp32r), gate multiply, transpose ----
            nc.sync.dma_start(identity[:], iden.ap())
            with tc.tile_pool(name="p1b", bufs=2) as p1b:
                axps = []
                for t in range(TT):
                    ap_t = psum.tile(
                        [P, 512], F32, tag="bank", name=f"axps{t}"
                    )
                    axps.append(ap_t)
                for k in range(KT):
                    lak = p1b.tile([P, 512], F32R, tag="lak", name="lak", bufs=4)
                    nc.sync.dma_start(lak[:], laT3[:, k, :])
                    for t in range(TT):
                        nc.tensor.matmul(
                            axps[t][:], xsb[:, k, ts(t, P)], lak[:],
                            start=(k == 0), stop=(k == KT - 1),
                        )
                axws = []
                for t in range(TT):
                    axw = p1b.tile(
                        [P, 512], F32R, tag=f"axw{t}", name=f"axw{t}", bufs=1
                    )
                    nc.vector.tensor_tensor(
                        axw[:].rearrange("p (e r) -> p e r", r=R),
                        axps[t][:].rearrange("p (e r) -> p e r", r=R),
                        wdense[t][:, :, None].to_broadcast([P, E, R]),
                        mybir.AluOpType.mult,
                    )
                    axws.append(axw)
                for t in range(TT):
                    tpq = psum.tile([P, 512], F32R, tag="bank", name="tpq")
                    for rr in range(RR):
                        nc.tensor.transpose(
                            tpq[:, ts(rr, P)], axws[t][:, ts(rr, P)],
                            identity[:],
                        )
                    nc.vector.tensor_copy(
                        axwT[:, :, ts(t, P)],
                        tpq[:].rearrange("p (rr q) -> p rr q", q=P),
                    )

            # ---- phase 2: base + delta per output tile ----
            KPC = KT // KC  # k-slices per base-weight chunk
            with (
                tc.tile_pool(name="p2bw", bufs=6) as p2bw,
                tc.tile_pool(name="p2lb", bufs=3) as p2lb,
                tc.tile_pool(name="p2o", bufs=4) as p2o,
            ):

                def load_lb(o):
                    lb = p2lb.tile([P, RR, 512], F32R, tag="lb", name="lb")
                    nc.sync.dma_start(lb[:], lbT3[:, :, ds(o * 512, 512)])
                    return lb

                def load_bwc(o, kc):
                    bwc = p2bw.tile([P, KPC, 512], F32R, tag="bwc", name="bwc")
                    nc.sync.dma_start(
                        bwc[:],
                        bwT2[
                            ds(kc * KPC * P, KPC * P), ds(o * 512, 512)
                        ].rearrange("(kk p) o -> p kk o", p=P),
                    )
                    return bwc

                lb_next = load_lb(0)
                bw_pre = {0: load_bwc(0, 0), 1: load_bwc(0, 1)}
                for o in range(OT):
                    lb = lb_next
                    ps2 = {}
                    for kc in range(KC):
                        bwc = bw_pre.pop(kc, None)
                        if bwc is None:
                            bwc = load_bwc(o, kc)
                        for t in range(TT):
                            if kc == 0:
                                ps2[t] = psum.tile(
                                    [P, 512], F32, tag="bank",
                                    name=f"ps2_{o}_{t}",
                                )
                            for k in range(KPC):
                                nc.tensor.matmul(
                                    ps2[t][:],
                                    xsb[:, kc * KPC + k, ts(t, P)],
                                    bwc[:, k, :],
                                    start=(kc == 0 and k == 0),
                                    stop=False,
                                )
                    # prefetch next o ahead of this o's output burst
                    if o + 1 < OT:
                        lb_next = load_lb(o + 1)
                        bw_pre = {
                            0: load_bwc(o + 1, 0),
                            1: load_bwc(o + 1, 1),
                        }
                    for t in range(TT):
                        for rr in range(RR):
                            nc.tensor.matmul(
                                ps2[t][:],
                                axwT[:, rr, ts(t, P)],
                                lb[:, rr, :],
                                start=False,
                                stop=(rr == RR - 1),
                            )
                        osb = p2o.tile([P, 512], F32, tag="osb", name="osb")
                        nc.vector.tensor_copy(osb[:], ps2[t][:])
                        nc.sync.dma_start(
                            out2[ts(t, P), ds(o * 512, 512)], osb[:]
                        )

    nc.compile()
    return nc


def _get_nc():
    if "nc" not in _CACHE:
        _CACHE["nc"] = _build()
    return _CACHE["nc"]


def kernel(x, base_w, gate_w, lora_A, lora_B):
    nc = _get_nc()

    x2 = np.ascontiguousarray(np.asarray(x, dtype=np.float32).reshape(B * S, DIN))
    bwT = np.ascontiguousarray(np.asarray(base_w, dtype=np.float32).T)
    gwT = np.ascontiguousarray(np.asarray(gate_w, dtype=np.float32).T)
    laT = np.ascontiguousarray(np.asarray(lora_A, dtype=np.float32).T)
    lbT = np.ascontiguousarray(
        np.asarray(lora_B, dtype=np.float32).T * np.float32(SCALING)
    )
    iden = np.eye(P, dtype=np.float32)

    in_maps = []
    for c in range(NCORES):
        xT_c = np.ascontiguousarray(x2[c * T : (c + 1) * T].T)
        in_maps.append(
            {
                "xT": xT_c,
                "bwT": bwT,
                "gwT": gwT,
                "laT": laT,
                "lbT": lbT,
                "iden": iden,
            }
        )

    res = bass_utils.run_bass_kernel_spmd(nc, in_maps, core_ids=list(range(NCORES)))
    parts = [res.results[c]["out"] for c in range(NCORES)]
    return np.concatenate(parts, axis=0).reshape(B, S, DOUT).astype(np.float32)
